# revision 1
# baseline (speedup 1.0000x reference)
"""LSTM decoder (teacher-forcing) kernel for Trainium2, 8 NeuronCores.

Sharding: vocab-tensor-parallel on the fc_out layer (V=32000 -> 4000/core);
the LSTM recurrence is replicated on every core (cheap vs fc), so there are
no collectives.  Each core computes logits[:, :, c*4000:(c+1)*4000] for all
(B=64, T=20) and the host concatenates.

Device kernel layout (per core), all matmuls bf16 with fp32 PSUM accum:
  - per 128-token block (= 2 timesteps): embedding rows gathered on-device
    via indirect DMA, cast to bf16, PE-transposed to embT, and the input-
    side gate contribution xpart = emb @ W_ih.T + bias precomputed at full
    128-partition PE utilization.  These chains are emitted interleaved
    with the recurrence (2 pairs ahead) so they pipeline as PE filler.
  - per step t: gates PSUM [64, 4H] is initialized with xpart_t via a K=64
    identity matmul (handles the odd-step upper-partition half without any
    cross-partition copies), then 4 W_hh k-tiles accumulate h @ W_hh.T.
    n-outer loop order so each gate block's bank finishes early.
  - gate blocks are reordered [i, g, f, o] and activated per-block so the
    c/h pointwise chain (fp32, ACT+DVE) starts before the last bank lands.
  - h cast to bf16 and PE-transposed (4x [64,128]) into the hT pair tile.
  - fc runs half-a-pair (4 of 8 n-tiles) per step at M=128 (two steps'
    hT stacked), emitted between gates and transposes so it fills PE
    during the pointwise tail; bias is added by the DVE drain; one 2MB
    DMA per pair writes the logits.
  - _split_excess_waits post-pass: this toolchain's walrus encodes at most
    one sync wait per instruction, so excess Tile-generated waits are moved
    onto same-engine EventSemaphore ops.
"""

from contextlib import ExitStack

import ml_dtypes
import numpy as np

import concourse.bass as bass
import concourse.mybir as mybir
import concourse.tile as tile
from concourse.bass_utils import run_bass_kernel_spmd

B, T, H, E, V = 64, 20, 512, 512, 32000
NC = 8
VS = V // NC  # 4000 vocab rows per core
BT = B * T  # 1280
G4 = 4 * H  # 2048
KT = (E + H) // 128  # 8 contraction k-tiles for the gates matmul
NPAIR = T // 2
NF = 8  # fc n-tiles per pair
FS = VS // NF  # 500
BF16 = ml_dtypes.bfloat16

f32 = mybir.dt.float32
bf16 = mybir.dt.bfloat16
i32 = mybir.dt.int32
Act = mybir.ActivationFunctionType


def _split_excess_waits(nc: bass.Bass) -> None:
    """Walrus codegen caps the number of sync-wait commands per instruction
    (1 for DIRECT2D DMAs, ~4 for Drain).  Move excess waits onto
    InstEventSemaphore ops emitted just before, on the same engine —
    semantically identical since the sequencer executes waits in order."""
    limit = 1
    n = 0
    for bb in nc.main_func.blocks:
        out = []
        for ins in bb.instructions:
            si = ins.sync_info
            if (
                si is not None
                and len(si.on_wait) > limit
                and getattr(ins, "opcode", None) != "EventSemaphore"
            ):
                waits = list(si.on_wait)
                excess, keep = waits[:-limit], waits[-limit:]
                for j in range(0, len(excess), 1):
                    ev = mybir.InstEventSemaphore(
                        name=f"{ins.name}-ws{n}",
                        ins=[],
                        outs=[],
                        sync_info=mybir.SyncInfo(
                            on_wait=excess[j : j + 1], on_update=[]
                        ),
                    )
                    ev.engine = ins.engine
                    out.append(ev)
                    n += 1
                si.on_wait = keep
            out.append(ins)
        bb.instructions = out


def build_bass(reps: int = 1) -> bass.Bass:
    nc = bass.Bass()

    embed_w = nc.dram_tensor("embed_w", [V, E], f32, kind="ExternalInput")
    idx_d = nc.dram_tensor("idx", [BT, 1], i32, kind="ExternalInput")
    wt_d = nc.dram_tensor("wt", [E + H, G4], bf16, kind="ExternalInput")
    brep_d = nc.dram_tensor("bias_rep", [128, G4], bf16, kind="ExternalInput")
    fcw_d = nc.dram_tensor("fc_wt", [H, VS], bf16, kind="ExternalInput")
    fcb_d = nc.dram_tensor("fc_b", [128, VS], bf16, kind="ExternalInput")
    xt_d = nc.dram_tensor("xt", [128, 4 * B], bf16, kind="ExternalInput")
    c0_d = nc.dram_tensor("c0", [B, H], f32, kind="ExternalInput")
    ident_d = nc.dram_tensor("ident", [128, 128], bf16, kind="ExternalInput")
    out_d = nc.dram_tensor("out", [BT, VS], f32, kind="ExternalOutput")

    with tile.TileContext(nc) as tc, ExitStack() as ctx:
        consts = ctx.enter_context(tc.tile_pool(name="consts", bufs=1))
        ps_g = ctx.enter_context(tc.tile_pool(name="ps_g", bufs=1, space="PSUM"))
        ps_f = ctx.enter_context(tc.tile_pool(name="ps_f", bufs=2, space="PSUM"))
        ps_e = ctx.enter_context(tc.tile_pool(name="ps_e", bufs=1, space="PSUM"))
        ps_h = ctx.enter_context(tc.tile_pool(name="ps_h", bufs=1, space="PSUM"))

        for _rep in range(reps):
            _emit_body(
                nc, tc, consts, ps_g, ps_f, ps_h, ps_e,
                embed_w, idx_d, wt_d, brep_d, fcw_d, fcb_d, xt_d, c0_d,
                ident_d, out_d,
            )

    _split_excess_waits(nc)
    return nc


def _emit_body(
    nc, tc, consts, ps_g, ps_f, ps_h, ps_e,
    embed_w, idx_d, wt_d, brep_d, fcw_d, fcb_d, xt_d, c0_d, ident_d,
    out_d,
):
    if True:  # keep indentation of the original body
        # ---- resident constants in SBUF ----
        wt_ks = []
        for k in range(KT):
            wk = consts.tile([128, G4], bf16, name=f"wt_sb", bufs=KT)
            nc.sync.dma_start(out=wk[:], in_=wt_d[k * 128 : (k + 1) * 128, :])
            wt_ks.append(wk)
        fcw_ks = []
        for k in range(4):
            fk = consts.tile([128, VS], bf16, name=f"fcw_sb", bufs=4)
            nc.sync.dma_start(out=fk[:], in_=fcw_d[k * 128 : (k + 1) * 128, :])
            fcw_ks.append(fk)
        fcb_sb = consts.tile([128, VS], bf16, name="fcb_sb")
        nc.sync.dma_start(out=fcb_sb[:], in_=fcb_d[:])
        xt_sb = consts.tile([128, 4 * B], bf16, name="xt_sb")
        nc.sync.dma_start(out=xt_sb[:], in_=xt_d[:])
        ident_sb = consts.tile([128, 128], bf16, name="ident_sb")
        nc.sync.dma_start(out=ident_sb[:], in_=ident_d[:])
        c_prev = consts.tile([B, H], f32, name="c0_sb")
        nc.sync.dma_start(out=c_prev[:], in_=c0_d[:])

        # ---- embedding gather (device-side), cast during gather, then
        # transpose on PE into embT (time-major lhsT layout) ----
        # all 1280 indices in one DMA: idx_all[p, i] = idx[i*128 + p]
        idx_all = consts.tile([128, BT // 128], i32, name="idx_all")
        nc.sync.dma_start(
            out=idx_all[:], in_=idx_d[:, 0].rearrange("(i p) -> p i", p=128)
        )
        ppool = tc.alloc_tile_pool(name="pointwise", bufs=1)
        hpool = tc.alloc_tile_pool(name="ht", bufs=3)
        opool = tc.alloc_tile_pool(name="outsb", bufs=2)
        gpool = tc.alloc_tile_pool(name="gather", bufs=2)
        brep_sb = gpool.tile([128, G4], bf16, name="brep_sb", bufs=1)
        nc.sync.dma_start(out=brep_sb[:], in_=brep_d[:])
        # per-block chain gather -> cast -> PE transpose -> xpart matmul ->
        # drain, emitted interleaved with the recurrence (2 pairs ahead) so
        # PE work pipelines instead of forming a serial preamble phase.
        xparts = []

        def emit_block(i):
            grow = gpool.tile([128, E], f32, name="grow")
            nc.gpsimd.indirect_dma_start(
                out=grow[:],
                out_offset=None,
                in_=embed_w[:],
                in_offset=bass.IndirectOffsetOnAxis(ap=idx_all[:, i : i + 1], axis=0),
            )
            grow_bf = gpool.tile([128, E], bf16, name="grow_bf")
            nc.vector.tensor_copy(out=grow_bf[:], in_=grow[:])
            e_ps = ps_e.tile([128, E], bf16, name="e_ps")
            for k in range(4):
                nc.tensor.transpose(
                    out=e_ps[:, k * 128 : (k + 1) * 128],
                    in_=grow_bf[:, k * 128 : (k + 1) * 128],
                    identity=ident_sb[:],
                )
            embT_i = gpool.tile([128, E], bf16, name="embT_i")
            nc.vector.tensor_copy(out=embT_i[:], in_=e_ps[:])
            # xpart[i] = emb_pair_i @ W_ih.T + bias, full 128 partitions.
            # psx shares PSUM banks with the gates matmul, so drain each
            # bank right after its own 4 matmuls — a monolithic drain would
            # stall the next gates inject ~1.5us on every even step.
            psx = ps_g.tile([128, G4], f32, name="g_ps")
            xp = consts.tile([128, G4], bf16, name="xpart", bufs=BT // 128)
            for n in range(4):
                for k in range(4):
                    nc.tensor.matmul(
                        out=psx[:, n * 512 : (n + 1) * 512],
                        lhsT=embT_i[:, k * 128 : (k + 1) * 128],
                        rhs=wt_ks[k][:, n * 512 : (n + 1) * 512],
                        start=(k == 0),
                        stop=(k == 3),
                    )
                nc.vector.tensor_add(
                    out=xp[:, n * 512 : (n + 1) * 512],
                    in0=psx[:, n * 512 : (n + 1) * 512],
                    in1=brep_sb[:, n * 512 : (n + 1) * 512],
                )
            xparts.append(xp)

        emit_block(0)
        emit_block(1)

        # ---- recurrence ----
        hT_tiles = []
        fc_state = {}

        def emit_fc_chunk(u):
            _emit_fc_chunk(
                nc, opool, ps_f, hT_tiles, fcw_ks, fcb_sb, out_d, fc_state, u
            )

        hT_pair = None
        for t in range(T):
            # prefetch the gather/xpart chain two pairs ahead
            if t % 2 == 0 and t // 2 + 2 < BT // 128:
                emit_block(t // 2 + 2)
            if t == T - 2:
                gpool.release()
            # gates matmul: PSUM initialized with xpart_t (+bias) via a K=64
            # identity matmul, then W_hh k-tiles accumulate the h part.
            j, half = t // 2, t % 2
            g_ps = ps_g.tile([B, G4], f32, name="g_ps")
            inj_lhs = ident_sb[half * B : (half + 1) * B, half * B : (half + 1) * B]
            # n-outer: each gate block's PSUM bank completes early so its
            # activation overlaps the remaining blocks' matmuls.
            for n in range(4):
                nc.tensor.matmul(
                    out=g_ps[:, n * 512 : (n + 1) * 512],
                    lhsT=inj_lhs,
                    rhs=xparts[j][
                        half * B : (half + 1) * B, n * 512 : (n + 1) * 512
                    ],
                    start=True,
                    stop=False,
                )
                for k in range(4):
                    if t == 0:
                        lhs = xt_sb[:, k * B : (k + 1) * B]
                    else:
                        lhs = hT_pair[
                            :,
                            k * 128 + ((t - 1) % 2) * B : k * 128
                            + ((t - 1) % 2 + 1) * B,
                        ]
                    nc.tensor.matmul(
                        out=g_ps[:, n * 512 : (n + 1) * 512],
                        lhsT=lhs,
                        rhs=wt_ks[4 + k][:, n * 512 : (n + 1) * 512],
                        start=False,
                        stop=(k == 3),
                    )

            # pointwise (fp32): gate blocks are [i, g, f, o] so the c-chain
            # starts as soon as the first two PSUM banks are done.
            gb = lambda m: g_ps[:, m * 512 : (m + 1) * 512]
            sig_i = ppool.tile([B, 512], f32, name="sig_i")
            nc.scalar.activation(out=sig_i[:], in_=gb(0), func=Act.Sigmoid)
            g_t = ppool.tile([B, 512], f32, name="g_t")
            nc.scalar.activation(out=g_t[:], in_=gb(1), func=Act.Tanh)
            ig = ppool.tile([B, 512], f32, name="ig")
            nc.vector.tensor_mul(out=ig[:], in0=sig_i[:], in1=g_t[:])
            sig_f = ppool.tile([B, 512], f32, name="sig_f")
            nc.scalar.activation(out=sig_f[:], in_=gb(2), func=Act.Sigmoid)
            fcs = ppool.tile([B, 512], f32, name="fcs")
            nc.vector.tensor_mul(out=fcs[:], in0=sig_f[:], in1=c_prev[:])
            c_new = ppool.tile([B, H], f32, name="c_new")
            nc.vector.tensor_add(out=c_new[:], in0=ig[:], in1=fcs[:])
            sig_o = ppool.tile([B, 512], f32, name="sig_o")
            nc.scalar.activation(out=sig_o[:], in_=gb(3), func=Act.Sigmoid)
            tc_t = ppool.tile([B, 512], f32, name="tc_t")
            nc.scalar.activation(out=tc_t[:], in_=c_new[:], func=Act.Tanh)
            h_bf = ppool.tile([B, H], bf16, name="h_bf", bufs=2)
            nc.vector.tensor_mul(out=h_bf[:], in0=sig_o[:], in1=tc_t[:])
            c_prev = c_new

            # fc chunk sits between gates and transposes in PE program order:
            # it fills the PE while the pointwise tail computes h.  Lag of 2
            # steps: chunk u reads hT pair u//2, complete after step u.
            if t >= 2:
                emit_fc_chunk(t - 2)

            # transpose h into hT slot (4x PE transpose + one DVE copy)
            if t % 2 == 0:
                hT_pair = hpool.tile([128, 512], bf16, name="hT_pair")
                hT_tiles.append(hT_pair)
            h_ps = ps_h.tile([128, 4 * B], bf16, name="h_ps")
            for k in range(4):
                nc.tensor.transpose(
                    out=h_ps[:, k * B : (k + 1) * B],
                    in_=h_bf[:, k * 128 : (k + 1) * 128],
                    identity=ident_sb[0:B, 0:B],
                )
            nc.vector.tensor_copy(
                out=hT_pair.rearrange("p (k c) -> p k c", k=4)[
                    :, :, (t % 2) * B : (t % 2 + 1) * B
                ],
                in_=h_ps.rearrange("p (k c) -> p k c", k=4),
            )
        emit_fc_chunk(T - 2)
        emit_fc_chunk(T - 1)
        opool.release()
        hpool.release()
        ppool.release()


def _emit_fc_chunk(
    nc, opool, ps_f, hT_tiles, fcw_ks, fcb_sb, out_d, state, u
):
    """Emit fc n-tiles [0:4) (u even) or [4:8) (u odd) for pair u//2."""
    pair = u // 2
    half = u % 2
    hT_pair = hT_tiles[pair]
    if half == 0:
        state["out_sb"] = opool.tile([128, VS], f32, name="out_sb")
    out_sb = state["out_sb"]
    for n in range(half * 4, half * 4 + 4):
        f_ps = ps_f.tile([128, FS], f32, name="f_ps")
        for k in range(4):
            nc.tensor.matmul(
                out=f_ps[:],
                lhsT=hT_pair[:, k * 128 : (k + 1) * 128],
                rhs=fcw_ks[k][:, n * FS : (n + 1) * FS],
                start=(k == 0),
                stop=(k == 3),
            )
        nc.vector.tensor_add(
            out=out_sb[:, n * FS : (n + 1) * FS],
            in0=f_ps[:],
            in1=fcb_sb[:, n * FS : (n + 1) * FS],
        )
    if half == 1:
        nc.sync.dma_start(
            out=out_d[pair * 128 : (pair + 1) * 128, :], in_=out_sb[:]
        )


def build_null() -> bass.Bass:
    """Same I/O signature, near-empty body — measures dispatch overhead."""
    nc = bass.Bass()
    nc.dram_tensor("embed_w", [V, E], f32, kind="ExternalInput")
    nc.dram_tensor("idx", [BT, 1], i32, kind="ExternalInput")
    nc.dram_tensor("wt", [E + H, G4], bf16, kind="ExternalInput")
    nc.dram_tensor("bias_rep", [128, G4], bf16, kind="ExternalInput")
    nc.dram_tensor("fc_wt", [H, VS], bf16, kind="ExternalInput")
    nc.dram_tensor("fc_b", [128, VS], bf16, kind="ExternalInput")
    nc.dram_tensor("xt", [128, 4 * B], bf16, kind="ExternalInput")
    c0_d = nc.dram_tensor("c0", [B, H], f32, kind="ExternalInput")
    nc.dram_tensor("ident", [128, 128], bf16, kind="ExternalInput")
    out_d = nc.dram_tensor("out", [BT, VS], f32, kind="ExternalOutput")
    with tile.TileContext(nc) as tc, ExitStack() as ctx:
        pool = ctx.enter_context(tc.tile_pool(name="p", bufs=1))
        t = pool.tile([B, H], f32, name="t")
        nc.sync.dma_start(out=t[:], in_=c0_d[:])
        nc.sync.dma_start(out=out_d[0:B, 0:H], in_=t[:])
    _split_excess_waits(nc)
    return nc


def _prep_inputs(x, captions, embed_w, W_ih, W_hh, b_ih, b_hh, fc_w, fc_b):
    """Host-side layout prep + sharding. Returns per-core input maps."""
    x = np.asarray(x, np.float32)
    captions = np.asarray(captions)
    embed_w = np.ascontiguousarray(np.asarray(embed_w, np.float32))
    W_ih = np.asarray(W_ih, np.float32)
    W_hh = np.asarray(W_hh, np.float32)
    b_ih = np.asarray(b_ih, np.float32)
    b_hh = np.asarray(b_hh, np.float32)
    fc_w = np.asarray(fc_w, np.float32)
    fc_b = np.asarray(fc_b, np.float32)

    # gate-block permutation i,f,g,o -> i,g,f,o
    perm = np.concatenate(
        [np.arange(0, 512), np.arange(1024, 1536), np.arange(512, 1024),
         np.arange(1536, 2048)]
    )
    wcat = np.concatenate([W_ih, W_hh], axis=1)[perm]  # [4H, E+H]
    wt = np.ascontiguousarray(wcat.T).astype(BF16)  # [E+H, 4H]
    bias_rep = np.ascontiguousarray(
        np.broadcast_to((b_ih + b_hh)[perm], (128, G4))
    ).astype(BF16)
    idx = np.ascontiguousarray(captions.T).reshape(BT, 1).astype(np.int32)
    xt = np.ascontiguousarray(
        x.T.reshape(4, 128, B).transpose(1, 0, 2).reshape(128, 4 * B)
    ).astype(BF16)
    ident = np.eye(128, dtype=BF16)

    shared = {
        "embed_w": embed_w,
        "idx": idx,
        "wt": wt,
        "bias_rep": bias_rep,
        "xt": xt,
        "c0": np.ascontiguousarray(x),
        "ident": ident,
    }
    in_maps = []
    for c in range(NC):
        sl = slice(c * VS, (c + 1) * VS)
        m = dict(shared)
        m["fc_wt"] = np.ascontiguousarray(fc_w[sl].T).astype(BF16)
        m["fc_b"] = np.ascontiguousarray(
            np.broadcast_to(fc_b[sl], (128, VS))
        ).astype(BF16)
        in_maps.append(m)
    return in_maps


def _assemble(results):
    out = np.empty((B, T, V), np.float32)
    for c in range(NC):
        r = np.asarray(results[c]["out"]).reshape(T, B, VS)
        out[:, :, c * VS : (c + 1) * VS] = r.transpose(1, 0, 2)
    return out


def _run(inputs, trace=False, **kw):
    nc = build_bass()
    in_maps = _prep_inputs(**inputs)
    res = run_bass_kernel_spmd(nc, in_maps, core_ids=list(range(NC)), trace=trace, **kw)
    return _assemble(res.results), res


def kernel(**inputs) -> np.ndarray:
    return _run(inputs)[0]



# revision 6
# speedup vs baseline: 2.2251x; 2.2251x over previous
"""LSTM decoder (teacher-forcing) kernel for Trainium2, 8 NeuronCores.

Sharding: 2-way data parallel over batch x 4-way tensor parallel over vocab.
Each core runs the recurrence for its 32-sample batch half (replicated x4)
and computes logits for its 8000-column vocab quarter.  No collectives.

Device kernel (per core), all matmuls bf16 with fp32 PSUM accum:
  - gates matmul uses 4-way column tiling of the PE array: batch (M=32) in
    col-group q, streaming W columns for H-quarter q, so the full 128-wide
    array is busy despite the small batch.  Per step, one PSUM bank holds
    gates.T-free layout [ (q,b), i|f|o|g x 128 ]; the bias (K=1 ones
    matmul), embedding part (embT stationary) and h part (hT stationary)
    all accumulate into it directly - no separate xpart pipeline, no
    injects, no gate-drain DVE pass.  Each col-group's first matmul carries
    start=True (has_written clears per element range, verified on HW).
  - embedding rows are gathered on-device per 128-token block (= 4 steps),
    cast to bf16, PE-transposed into embT (E-major stationary layout).
  - pointwise runs on [128,*] tiles at full lane count: one sigmoid over
    i|f|o, one tanh(g), 4 DVE muls/adds, one tanh(c).
  - h.T falls out of ONE [128,128] PE transpose per step (col-group q of
    the result is hT k-tile q); a single DVE copy files it into the hT
    quad tile that both the next step's gates and the fc consume.
  - fc runs per quad (4 steps x 32 batch = M=128 tokens) at full PE width,
    n-tiles of 500 columns, interleaved 4 per step one quad behind the
    recurrence as PE filler; drains alternate DVE/ACT, output is bf16 and
    the fc bias is folded in on the host after the gather.
  - _split_excess_waits post-pass: walrus encodes at most one sync wait
    per instruction; excess Tile waits move onto same-engine EventSemaphore
    ops.
"""

from contextlib import ExitStack

import ml_dtypes
import numpy as np

import concourse.bass as bass
import concourse.mybir as mybir
import concourse.tile as tile
from concourse.bass_utils import run_bass_kernel_spmd

B, T, H, E, V = 64, 20, 512, 512, 32000
NC = 8
BHALF = 2  # batch-parallel ways
VQ = 4  # vocab-parallel ways
BL = B // BHALF  # 32 samples per core
VS = V // VQ  # 8000 vocab cols per core
NT = BL * T  # 640 tokens per core
SPB = 128 // BL  # 4 steps per 128-token block
NBLK = NT // 128  # 5 blocks
G4 = 4 * H  # 2048
FS = 500
NF = VS // FS  # 16 fc n-tiles per quad
BF16 = ml_dtypes.bfloat16

f32 = mybir.dt.float32
bf16 = mybir.dt.bfloat16
i32 = mybir.dt.int32
Act = mybir.ActivationFunctionType


def _split_excess_waits(nc: bass.Bass) -> None:
    """Walrus codegen caps sync-wait commands per instruction (1 for
    DIRECT2D DMAs).  Move excess waits onto InstEventSemaphore ops emitted
    just before, on the same engine - semantically identical since the
    sequencer executes waits in order."""
    limit = 1
    n = 0
    for bb in nc.main_func.blocks:
        out = []
        for ins in bb.instructions:
            si = ins.sync_info
            if (
                si is not None
                and len(si.on_wait) > limit
                and getattr(ins, "opcode", None) != "EventSemaphore"
            ):
                waits = list(si.on_wait)
                excess, keep = waits[:-limit], waits[-limit:]
                for j in range(0, len(excess), 1):
                    ev = mybir.InstEventSemaphore(
                        name=f"{ins.name}-ws{n}",
                        ins=[],
                        outs=[],
                        sync_info=mybir.SyncInfo(
                            on_wait=excess[j : j + 1], on_update=[]
                        ),
                    )
                    ev.engine = ins.engine
                    out.append(ev)
                    n += 1
                si.on_wait = keep
            out.append(ins)
        bb.instructions = out


def build_bass(reps: int = 1) -> bass.Bass:
    nc = bass.Bass()

    d = {
        "embed_w": nc.dram_tensor("embed_w", [V, E], f32, kind="ExternalInput"),
        "idx": nc.dram_tensor("idx", [NT, 1], i32, kind="ExternalInput"),
        "wt": nc.dram_tensor("wt", [E + H, G4], bf16, kind="ExternalInput"),
        "brow": nc.dram_tensor("brow", [1, G4], bf16, kind="ExternalInput"),
        "ones": nc.dram_tensor("ones", [1, BL], bf16, kind="ExternalInput"),
        "fcw": nc.dram_tensor("fcw", [H, VS], bf16, kind="ExternalInput"),
        "xt": nc.dram_tensor("xt", [128, 128], bf16, kind="ExternalInput"),
        "c0": nc.dram_tensor("c0", [128, 128], f32, kind="ExternalInput"),
        "ident": nc.dram_tensor("ident", [128, 128], bf16, kind="ExternalInput"),
        "out": nc.dram_tensor("out", [NT, VS], bf16, kind="ExternalOutput"),
    }

    with tile.TileContext(nc) as tc, ExitStack() as ctx:
        consts = ctx.enter_context(tc.tile_pool(name="consts", bufs=1))
        ps_g = ctx.enter_context(tc.tile_pool(name="ps_g", bufs=3, space="PSUM"))
        ps_f = ctx.enter_context(tc.tile_pool(name="ps_f", bufs=2, space="PSUM"))
        ps_e = ctx.enter_context(tc.tile_pool(name="ps_e", bufs=1, space="PSUM"))
        ps_h = ctx.enter_context(tc.tile_pool(name="ps_h", bufs=1, space="PSUM"))

        for _rep in range(reps):
            _emit_body(nc, tc, consts, ps_g, ps_f, ps_e, ps_h, d)

    _split_excess_waits(nc)
    return nc


def _emit_body(nc, tc, consts, ps_g, ps_f, ps_e, ps_h, d):
    # ---- resident constants ----
    wt_sb = consts.tile([128, 8 * G4], bf16, name="wt_sb")
    for k in range(8):
        nc.sync.dma_start(
            out=wt_sb[:, k * G4 : (k + 1) * G4],
            in_=d["wt"][k * 128 : (k + 1) * 128, :],
        )
    brow_sb = consts.tile([1, G4], bf16, name="brow_sb")
    nc.sync.dma_start(out=brow_sb[:], in_=d["brow"][:])
    ones_sb = consts.tile([1, BL], bf16, name="ones_sb")
    nc.sync.dma_start(out=ones_sb[:], in_=d["ones"][:])
    ident_sb = consts.tile([128, 128], bf16, name="ident_sb")
    nc.sync.dma_start(out=ident_sb[:], in_=d["ident"][:])
    xt_sb = consts.tile([128, 128], bf16, name="xt_sb")
    nc.sync.dma_start(out=xt_sb[:], in_=d["xt"][:])
    c0_sb = consts.tile([128, 128], f32, name="c0_sb")
    nc.sync.dma_start(out=c0_sb[:], in_=d["c0"][:])
    idx_all = consts.tile([128, NBLK], i32, name="idx_all")
    nc.sync.dma_start(
        out=idx_all[:], in_=d["idx"][:, 0].rearrange("(j p) -> p j", p=128)
    )
    # fcw k-tiles loaded lazily (k-th tile needed only once fc starts)
    fcw_sb = consts.tile([128, 4 * VS], bf16, name="fcw_sb")

    gpool = tc.alloc_tile_pool(name="gather", bufs=2)
    epool = tc.alloc_tile_pool(name="embT", bufs=3)
    hpool = tc.alloc_tile_pool(name="hT", bufs=3)
    pw = tc.alloc_tile_pool(name="pw", bufs=2)
    cpool = tc.alloc_tile_pool(name="c", bufs=2)
    opool = tc.alloc_tile_pool(name="outsb", bufs=2)

    embTs = []
    hTs = []
    fc_out = {}
    fcq = []

    def emit_gather(j):
        grow = gpool.tile([128, E], f32, name="grow")
        nc.gpsimd.indirect_dma_start(
            out=grow[:],
            out_offset=None,
            in_=d["embed_w"][:],
            in_offset=bass.IndirectOffsetOnAxis(ap=idx_all[:, j : j + 1], axis=0),
        )
        grow_bf = gpool.tile([128, E], bf16, name="grow_bf")
        nc.vector.tensor_copy(out=grow_bf[:], in_=grow[:])
        e_ps = ps_e.tile([128, E], bf16, name="e_ps")
        for k in range(4):
            nc.tensor.transpose(
                out=e_ps[:, k * 128 : (k + 1) * 128],
                in_=grow_bf[:, k * 128 : (k + 1) * 128],
                identity=ident_sb[:],
            )
        embT = epool.tile([128, E], bf16, name="embT")
        nc.vector.tensor_copy(out=embT[:], in_=e_ps[:])
        embTs.append(embT)

    def emit_fc(nmax):
        for _ in range(nmax):
            if not fcq:
                return
            jq, n = fcq.pop(0)
            if n == 0:
                fc_out[jq] = opool.tile([128, VS], bf16, name="out_sb")
            f_ps = ps_f.tile([128, FS], f32, name="f_ps")
            hTq = hTs[jq]
            for k in range(4):
                nc.tensor.matmul(
                    out=f_ps[:],
                    lhsT=hTq[:, k * 128 : (k + 1) * 128],
                    rhs=fcw_sb[:, k * VS + n * FS : k * VS + (n + 1) * FS],
                    start=(k == 0),
                    stop=(k == 3),
                )
            dst = fc_out[jq][:, n * FS : (n + 1) * FS]
            if n % 2 == 0:
                nc.vector.tensor_copy(out=dst, in_=f_ps[:])
            else:
                nc.scalar.copy(out=dst, in_=f_ps[:])
            if n == NF - 1:
                nc.sync.dma_start(
                    out=d["out"][jq * 128 : (jq + 1) * 128, :],
                    in_=fc_out[jq][:],
                )

    emit_gather(0)
    emit_gather(1)

    c_prev = c0_sb
    for t in range(T):
        j, tl = t // SPB, t % SPB
        if tl == 0 and j + 2 < NBLK:
            emit_gather(j + 2)
        if t < 4:
            # stream the fcw k-tile needed from step 4 on; spreads the 8MB
            # load so it doesn't fight the wt/gather preamble for HBM
            nc.sync.dma_start(
                out=fcw_sb[:, t * VS : (t + 1) * VS],
                in_=d["fcw"][t * 128 : (t + 1) * 128, :],
            )

        # ---- gates: bias + emb-part + h-part accumulate in one bank ----
        g_ps = ps_g.tile([128, 512], f32, name="g_ps")
        for q in range(4):
            nc.tensor.matmul(
                out=g_ps[32 * q : 32 * q + 32, :],
                lhsT=ones_sb[0:1, :],
                rhs=brow_sb[0:1, q * 512 : (q + 1) * 512],
                start=True,
                stop=False,
                tile_position=(0, 32 * q),
            )
        embT = embTs[j]
        for k in range(4):
            lhs = embT[:, k * 128 + tl * BL : k * 128 + (tl + 1) * BL]
            for q in range(4):
                nc.tensor.matmul(
                    out=g_ps[32 * q : 32 * q + 32, :],
                    lhsT=lhs,
                    rhs=wt_sb[:, k * G4 + q * 512 : k * G4 + (q + 1) * 512],
                    start=False,
                    stop=False,
                    tile_position=(0, 32 * q),
                )
        for k in range(4):
            if t == 0:
                lhs = xt_sb[:, k * BL : (k + 1) * BL]
            else:
                tp, tpl = (t - 1) // SPB, (t - 1) % SPB
                lhs = hTs[tp][:, k * 128 + tpl * BL : k * 128 + (tpl + 1) * BL]
            for q in range(4):
                nc.tensor.matmul(
                    out=g_ps[32 * q : 32 * q + 32, :],
                    lhsT=lhs,
                    rhs=wt_sb[
                        :, (4 + k) * G4 + q * 512 : (4 + k) * G4 + (q + 1) * 512
                    ],
                    start=False,
                    stop=(k == 3 and q == 3),
                    tile_position=(0, 32 * q),
                )

        # ---- pointwise: layout [ (q,b), i|f|o|g x 128 ] ----
        s_ifo = pw.tile([128, 384], f32, name="s_ifo")
        nc.scalar.activation(out=s_ifo[:], in_=g_ps[:, 0:384], func=Act.Sigmoid)
        t_g = pw.tile([128, 128], f32, name="t_g")
        nc.scalar.activation(out=t_g[:], in_=g_ps[:, 384:512], func=Act.Tanh)
        ig = pw.tile([128, 128], f32, name="ig")
        nc.vector.tensor_mul(out=ig[:], in0=s_ifo[:, 0:128], in1=t_g[:])
        fc_ = pw.tile([128, 128], f32, name="fcs")
        nc.vector.tensor_mul(out=fc_[:], in0=s_ifo[:, 128:256], in1=c_prev[:])
        c_new = cpool.tile([128, 128], f32, name="c_new")
        nc.vector.tensor_add(out=c_new[:], in0=ig[:], in1=fc_[:])
        t_c = pw.tile([128, 128], f32, name="t_c")
        nc.scalar.activation(out=t_c[:], in_=c_new[:], func=Act.Tanh)
        h_bf = pw.tile([128, 128], bf16, name="h_bf")
        nc.vector.tensor_mul(out=h_bf[:], in0=s_ifo[:, 256:384], in1=t_c[:])
        c_prev = c_new

        # fc filler between the gates and the h transpose in PE order
        emit_fc(4)

        # ---- h.T via one PE transpose; col-group q = hT k-tile q ----
        if tl == 0:
            hTs.append(hpool.tile([128, 512], bf16, name="hT"))
        h_ps = ps_h.tile([128, 128], bf16, name="h_ps")
        nc.tensor.transpose(out=h_ps[:], in_=h_bf[:], identity=ident_sb[:])
        nc.vector.tensor_copy(
            out=hTs[j].rearrange("p (k s b) -> p k s b", k=4, s=SPB)[:, :, tl, :],
            in_=h_ps.rearrange("p (q b) -> p q b", q=4),
        )
        if tl == SPB - 1:
            fcq.extend((j, n) for n in range(NF))

    emit_fc(len(fcq))
    opool.release()
    cpool.release()
    pw.release()
    hpool.release()
    epool.release()
    gpool.release()


def _prep_inputs(x, captions, embed_w, W_ih, W_hh, b_ih, b_hh, fc_w, fc_b):
    """Host-side layout prep + sharding. Returns per-core input maps."""
    x = np.asarray(x, np.float32)
    captions = np.asarray(captions)
    embed_w = np.ascontiguousarray(np.asarray(embed_w, np.float32))
    W_ih = np.asarray(W_ih, np.float32)
    W_hh = np.asarray(W_hh, np.float32)
    b_ih = np.asarray(b_ih, np.float32)
    b_hh = np.asarray(b_hh, np.float32)
    fc_w = np.asarray(fc_w, np.float32)

    # gates column layout: col q*512 + s*128 + r  <->  W row base_s + q*128 + r
    # with blocks ordered [i, f, o, g]  (orig rows: i 0:512, f 512:1024,
    # g 1024:1536, o 1536:2048)
    perm = np.concatenate(
        [
            base + q * 128 + np.arange(128)
            for q in range(4)
            for base in (0, 512, 1536, 1024)
        ]
    )
    wcat = np.concatenate([W_ih, W_hh], axis=1)[perm]  # [2048, E+H]
    wt = np.ascontiguousarray(wcat.T).astype(BF16)  # [E+H, 2048]
    brow = np.ascontiguousarray((b_ih + b_hh)[perm][None, :]).astype(BF16)
    ones = np.ones((1, BL), BF16)
    ident = np.eye(128, dtype=BF16)

    shared = {"embed_w": embed_w, "wt": wt, "brow": brow, "ones": ones,
              "ident": ident}
    per_bh = []
    for bh in range(BHALF):
        xh = x[bh * BL : (bh + 1) * BL]  # [32, 512]
        xt = np.ascontiguousarray(
            xh.T.reshape(4, 128, BL).transpose(1, 0, 2).reshape(128, 128)
        ).astype(BF16)
        c0 = np.ascontiguousarray(
            xh.reshape(BL, 4, 128).transpose(1, 0, 2).reshape(128, 128)
        ).astype(np.float32)
        idx = np.ascontiguousarray(
            captions[bh * BL : (bh + 1) * BL, :].T.reshape(NT, 1)
        ).astype(np.int32)
        per_bh.append({"xt": xt, "c0": c0, "idx": idx})
    in_maps = []
    for c in range(NC):
        bh, vq = c // VQ, c % VQ
        m = dict(shared)
        m.update(per_bh[bh])
        m["fcw"] = np.ascontiguousarray(
            fc_w[vq * VS : (vq + 1) * VS].T
        ).astype(BF16)
        in_maps.append(m)
    return in_maps


def _assemble(results, fc_b):
    out = np.empty((B, T, V), np.float32)
    for c in range(NC):
        bh, vq = c // VQ, c % VQ
        r = np.asarray(results[c]["out"]).astype(np.float32)
        r += fc_b[vq * VS : (vq + 1) * VS][None, :]
        r = r.reshape(T, BL, VS).transpose(1, 0, 2)
        out[bh * BL : (bh + 1) * BL, :, vq * VS : (vq + 1) * VS] = r
    return out


def _run(inputs, trace=False, **kw):
    nc = build_bass()
    in_maps = _prep_inputs(**inputs)
    res = run_bass_kernel_spmd(
        nc, in_maps, core_ids=list(range(NC)), trace=trace, **kw
    )
    fc_b = np.asarray(inputs["fc_b"], np.float32)
    return _assemble(res.results, fc_b), res


def kernel(**inputs) -> np.ndarray:
    return _run(inputs)[0]
